# revision 6
# baseline (speedup 1.0000x reference)
"""CoefficientMaxPool Trainium2 kernel (8-core data-parallel).

Problem: x [32, 512, 16, 128] f32.  Irreps group into degree blocks
l=0:[0,1), l=1:[1,4), l=2:[4,9), l=3:[9,16).  Per (batch, l, channel):
find the neighbor n* maximizing the degree-block squared norm, output
that neighbor's block components -> out [32, 16, 128].

Per core (4 batches), per batch:
  - DMA x[b] as [p=128, a=4, i=16, c=128] (n = a*128 + p)
  - ACT: x2 = x*x
  - GPSIMD: norms l1 (adds); DVE: norms l2/l3 (grouped reduces)
  - DVE: amax = max over a (tensor_tensor max tree)
  - PE: transpose amax -> [c, l, p] PSUM; DVE reduce-max over p -> maxv[c,l];
        PE transpose -> [1, l, c]; K=1 ones matmul broadcasts to maxB[p,l,c]
  - DVE: mask = (norms == maxB)  (exact equality, unique winner - verified)
  - DVE (l0..l2) + GPSIMD (l3): x *= mask[l(i)]  in place
  - PE: ones[128,1]^T @ x accumulated over a -> out[1, i*c] in PSUM
  - ACT copy PSUM->SBUF, DMA out.
"""

import os
import sys

import numpy as np

for _p in ("/opt/trn_rl_repo", "/opt/pypackages"):
    if _p not in sys.path:
        sys.path.append(_p)

from contextlib import ExitStack

import concourse.bacc as bacc
import concourse.bass as bass
import concourse.tile as tile
from concourse import mybir

N_CORES = 8
B_FULL, N, IRR, C = 32, 512, 16, 128
B = B_FULL // N_CORES  # 4 batches per core
P = 128                # partitions (n within chunk)
A = N // P             # 4 neighbor chunks
BLOCKS = [(0, 1), (1, 4), (4, 9), (9, 16)]  # irrep ranges per degree l
F32 = mybir.dt.float32
ADD = mybir.AluOpType.add
MAX = mybir.AluOpType.max
MULT = mybir.AluOpType.mult
EQ = mybir.AluOpType.is_equal

_cache = {}


def _build_bass():
    nc = bacc.Bacc("TRN2", target_bir_lowering=False, debug=False,
                   num_devices=N_CORES)
    x_in = nc.dram_tensor("x", [B, N, IRR, C], F32, kind="ExternalInput")
    out_t = nc.dram_tensor("out", [B, IRR, C], F32, kind="ExternalOutput")
    ident_d = nc.inline_tensor(np.eye(P, dtype=np.float32), name="ident")

    with tile.TileContext(nc) as tc, ExitStack() as ctx:
        # DRAM view: n = a*P + p  ->  [b, p, a, i, c]
        x_v = x_in.ap().rearrange("b (a p) i c -> b p a i c", p=P)
        out_v = out_t.ap().rearrange("b i c -> (b i c)").unsqueeze(0)

        xp = ctx.enter_context(tc.tile_pool(name="xp", bufs=2))
        x2p = ctx.enter_context(tc.tile_pool(name="x2p", bufs=2))
        med = ctx.enter_context(tc.tile_pool(name="med", bufs=2))
        outp = ctx.enter_context(tc.tile_pool(name="outp", bufs=2))
        singles = ctx.enter_context(tc.tile_pool(name="singles", bufs=1))
        pmax = ctx.enter_context(tc.tile_pool(name="pmax", bufs=1, space="PSUM"))
        pout = ctx.enter_context(tc.tile_pool(name="pout", bufs=1, space="PSUM"))

        ones = singles.tile([P, 1], F32)
        nc.vector.memset(ones, 1.0)
        onesrow = singles.tile([1, P], F32)
        nc.vector.memset(onesrow, 1.0)
        ident = singles.tile([P, P], F32)
        nc.sync.dma_start(out=ident, in_=ident_d.ap())

        for b in range(B):
            X = xp.tile([P, A, IRR, C], F32, tag="X")
            nc.sync.dma_start(out=X, in_=x_v[b])

            X2 = x2p.tile([P, A, IRR, C], F32, tag="X2")
            nc.scalar.activation(X2, X, mybir.ActivationFunctionType.Square)

            # grouped sums over i -> norms [P, A, 3, C] for l=1..3
            # (l=0 norm is X2[:, :, 0, :] itself)
            norms = med.tile([P, A, 3, C], F32, tag="norms")
            # l1 on GPSIMD: two adds
            nc.gpsimd.tensor_tensor(
                norms[:, :, 0, :], X2[:, :, 1, :], X2[:, :, 2, :], ADD)
            nc.gpsimd.tensor_tensor(
                norms[:, :, 0, :], norms[:, :, 0, :], X2[:, :, 3, :], ADD)
            # l2, l3 on DVE: grouped reduces (i innermost via AP)
            for j, (s, e) in ((1, BLOCKS[2]), (2, BLOCKS[3])):
                nc.vector.tensor_reduce(
                    out=norms[:, :, j, :],
                    in_=X2[:, :, s:e, :].rearrange("p a i c -> p a c i"),
                    axis=mybir.AxisListType.X,
                    op=ADD,
                )

            # max over a -> amax [P, 4, C] via tensor_tensor max trees
            amax = med.tile([P, 4, C], F32, tag="amax")
            t0 = med.tile([P, C], F32, tag="t0")
            nc.vector.tensor_tensor(t0, X2[:, 0, 0, :], X2[:, 1, 0, :], MAX)
            nc.vector.tensor_tensor(
                amax[:, 0, :], X2[:, 2, 0, :], X2[:, 3, 0, :], MAX)
            nc.vector.tensor_tensor(amax[:, 0, :], amax[:, 0, :], t0, MAX)
            t1 = med.tile([P, 3, C], F32, tag="t1")
            nc.vector.tensor_tensor(t1, norms[:, 0], norms[:, 1], MAX)
            nc.vector.tensor_tensor(amax[:, 1:4, :], norms[:, 2], norms[:, 3], MAX)
            nc.vector.tensor_tensor(amax[:, 1:4, :], amax[:, 1:4, :], t1, MAX)

            # cross-partition max: transpose [p, c] -> [c, p] per l, reduce
            nt = pmax.tile([P, 4, P], F32, tag="nt")  # [c, l, p]
            for l in range(4):
                nc.tensor.transpose(nt[:, l, :], amax[:, l, :], ident)
            maxv = med.tile([P, 4], F32, tag="maxv")  # [c, l]
            nc.vector.tensor_reduce(
                out=maxv, in_=nt, axis=mybir.AxisListType.X, op=MAX)
            # [c, l] -> [1, l, c]; broadcast to all partitions via K=1 matmul
            mvt = pmax.tile([1, 4, P], F32, tag="mvt")
            for l in range(4):
                nc.tensor.transpose(mvt[:, l, :], maxv[:, l:l + 1], ident)
            mvts = med.tile([1, 4, P], F32, tag="mvts")
            nc.scalar.copy(out=mvts, in_=mvt)
            maxB = pmax.tile([P, 4, C], F32, tag="maxB")  # [p(all), l, c]
            for l in range(4):
                nc.tensor.matmul(
                    maxB[:, l, :], onesrow, mvts[:, l, :],
                    start=True, stop=True,
                )

            # winner mask (exact equality; unique winner)
            mask = med.tile([P, A, 4, C], F32, tag="mask")
            nc.vector.tensor_tensor(
                mask[:, :, 0, :],
                X2[:, :, 0, :],
                maxB[:, 0:1, :].broadcast_to([P, A, C]),
                EQ,
            )
            nc.vector.tensor_tensor(
                mask[:, :, 1:4, :],
                norms,
                maxB[:, 1:4, :].unsqueeze(1).broadcast_to([P, A, 3, C]),
                EQ,
            )

            # select winner values in place: X *= mask[l(i)]
            # l0..l2 on DVE, l3 (the largest) on GPSIMD
            for l, (s, e) in enumerate(BLOCKS[:3]):
                nc.vector.tensor_tensor(
                    X[:, :, s:e, :],
                    X[:, :, s:e, :],
                    mask[:, :, l, :].unsqueeze(2).broadcast_to([P, A, e - s, C]),
                    MULT,
                )
            s, e = BLOCKS[3]
            nc.gpsimd.tensor_tensor(
                X[:, :, s:e, :],
                X[:, :, s:e, :],
                mask[:, :, 3, :].unsqueeze(2).broadcast_to([P, A, e - s, C]),
                MULT,
            )

            # sum over n (partitions via PE, chunks via PSUM accumulation)
            ps = pout.tile([1, 4, 512], F32, tag="ps")
            Xf = X.rearrange("p a i c -> p a (i c)")
            for k in range(4):
                for a in range(A):
                    nc.tensor.matmul(
                        ps[:, k, :],
                        ones,
                        Xf[:, a, k * 512:(k + 1) * 512],
                        start=(a == 0),
                        stop=(a == A - 1),
                    )

            ob = outp.tile([1, IRR * C], F32, tag="ob")
            nc.scalar.copy(out=ob, in_=ps.rearrange("m k f -> m (k f)"))
            nc.sync.dma_start(out=out_v[:, b * IRR * C:(b + 1) * IRR * C], in_=ob)

    nc.compile()
    return nc


def kernel(x: np.ndarray, i2l: np.ndarray | None = None) -> np.ndarray:
    x = np.ascontiguousarray(np.asarray(x), dtype=np.float32)
    assert x.shape == (B_FULL, N, IRR, C), x.shape

    if "nc" not in _cache:
        _cache["nc"] = _build_bass()
    nc = _cache["nc"]

    from concourse.bass_utils import run_bass_kernel_spmd

    in_maps = [{"x": x[i * B:(i + 1) * B]} for i in range(N_CORES)]
    res = run_bass_kernel_spmd(nc, in_maps, list(range(N_CORES)))
    out = np.concatenate([res.results[i]["out"] for i in range(N_CORES)], axis=0)
    return out


if __name__ == "__main__":
    xs = np.random.randn(B_FULL, N, IRR, C).astype(np.float32)
    o = kernel(xs)
    print("out", o.shape, o.dtype)


# revision 7
# speedup vs baseline: 1.0105x; 1.0105x over previous
"""CoefficientMaxPool Trainium2 kernel (8-core data-parallel).

Problem: x [32, 512, 16, 128] f32.  Irreps group into degree blocks
l=0:[0,1), l=1:[1,4), l=2:[4,9), l=3:[9,16).  Per (batch, l, channel):
find the neighbor n* maximizing the degree-block squared norm, output
that neighbor's block components -> out [32, 16, 128].

Per core (4 batches), per batch:
  - DMA x[b] as [p=128, a=4, i=16, c=128] (n = a*128 + p)
  - ACT: x2 = x*x
  - GPSIMD: norms l1 (adds); DVE: norms l2/l3 (grouped reduces)
  - DVE: amax = max over a (tensor_tensor max tree)
  - PE: transpose amax -> [c, l, p] PSUM; DVE reduce-max over p -> maxv[c,l];
        PE transpose -> [1, l, c]; K=1 ones matmul broadcasts to maxB[p,l,c]
  - DVE: mask = (norms == maxB)  (exact equality, unique winner - verified)
  - DVE (l0..l2) + GPSIMD (l3): x *= mask[l(i)]  in place
  - PE: ones[128,1]^T @ x accumulated over a -> out[1, i*c] in PSUM
  - ACT copy PSUM->SBUF, DMA out.
"""

import os
import sys

import numpy as np

for _p in ("/opt/trn_rl_repo", "/opt/pypackages"):
    if _p not in sys.path:
        sys.path.append(_p)

from contextlib import ExitStack

import concourse.bacc as bacc
import concourse.bass as bass
import concourse.tile as tile
from concourse import mybir

N_CORES = 8
B_FULL, N, IRR, C = 32, 512, 16, 128
B = B_FULL // N_CORES  # 4 batches per core
P = 128                # partitions (n within chunk)
A = N // P             # 4 neighbor chunks
BLOCKS = [(0, 1), (1, 4), (4, 9), (9, 16)]  # irrep ranges per degree l
F32 = mybir.dt.float32
ADD = mybir.AluOpType.add
MAX = mybir.AluOpType.max
MULT = mybir.AluOpType.mult
EQ = mybir.AluOpType.is_equal

_cache = {}


def _build_bass():
    nc = bacc.Bacc("TRN2", target_bir_lowering=False, debug=False,
                   num_devices=N_CORES)
    x_in = nc.dram_tensor("x", [B, N, IRR, C], F32, kind="ExternalInput")
    out_t = nc.dram_tensor("out", [B, IRR, C], F32, kind="ExternalOutput")
    ident_d = nc.inline_tensor(np.eye(P, dtype=np.float32), name="ident")

    with tile.TileContext(nc) as tc, ExitStack() as ctx:
        # DRAM view: n = a*P + p  ->  [b, p, a, i, c]
        x_v = x_in.ap().rearrange("b (a p) i c -> b p a i c", p=P)
        out_v = out_t.ap().rearrange("b i c -> (b i c)").unsqueeze(0)

        xp = ctx.enter_context(tc.tile_pool(name="xp", bufs=2))
        x2p = ctx.enter_context(tc.tile_pool(name="x2p", bufs=2))
        med = ctx.enter_context(tc.tile_pool(name="med", bufs=3))
        outp = ctx.enter_context(tc.tile_pool(name="outp", bufs=2))
        singles = ctx.enter_context(tc.tile_pool(name="singles", bufs=1))
        pmax = ctx.enter_context(tc.tile_pool(name="pmax", bufs=2, space="PSUM"))
        pout = ctx.enter_context(tc.tile_pool(name="pout", bufs=1, space="PSUM"))

        ones = singles.tile([P, 1], F32)
        nc.vector.memset(ones, 1.0)
        onesrow = singles.tile([1, P], F32)
        nc.vector.memset(onesrow, 1.0)
        ident = singles.tile([P, P], F32)
        nc.sync.dma_start(out=ident, in_=ident_d.ap())

        for b in range(B):
            X = xp.tile([P, A, IRR, C], F32, tag="X")
            nc.sync.dma_start(out=X, in_=x_v[b])

            X2 = x2p.tile([P, A, IRR, C], F32, tag="X2")
            nc.scalar.activation(X2, X, mybir.ActivationFunctionType.Square)

            # grouped sums over i -> norms [P, A, 3, C] for l=1..3
            # (l=0 norm is X2[:, :, 0, :] itself)
            norms = med.tile([P, A, 3, C], F32, tag="norms")
            # grouped reduces (i innermost via AP)
            for j, (s, e) in ((0, BLOCKS[1]), (1, BLOCKS[2]), (2, BLOCKS[3])):
                nc.vector.tensor_reduce(
                    out=norms[:, :, j, :],
                    in_=X2[:, :, s:e, :].rearrange("p a i c -> p a c i"),
                    axis=mybir.AxisListType.X,
                    op=ADD,
                )

            # max over a -> amax [P, 4, C] via tensor_tensor max trees
            amax = med.tile([P, 4, C], F32, tag="amax")
            t0 = med.tile([P, C], F32, tag="t0")
            nc.vector.tensor_tensor(t0, X2[:, 0, 0, :], X2[:, 1, 0, :], MAX)
            nc.vector.tensor_tensor(
                amax[:, 0, :], X2[:, 2, 0, :], X2[:, 3, 0, :], MAX)
            nc.vector.tensor_tensor(amax[:, 0, :], amax[:, 0, :], t0, MAX)
            t1 = med.tile([P, 3, C], F32, tag="t1")
            nc.vector.tensor_tensor(t1, norms[:, 0], norms[:, 1], MAX)
            nc.vector.tensor_tensor(amax[:, 1:4, :], norms[:, 2], norms[:, 3], MAX)
            nc.vector.tensor_tensor(amax[:, 1:4, :], amax[:, 1:4, :], t1, MAX)

            # cross-partition max: transpose [p, c] -> [c, p] per l, reduce
            nt = pmax.tile([P, 4, P], F32, tag="nt")  # [c, l, p]
            for l in range(4):
                nc.tensor.transpose(nt[:, l, :], amax[:, l, :], ident)
            maxv = med.tile([P, 4], F32, tag="maxv")  # [c, l]
            for l in range(4):
                nc.vector.tensor_reduce(
                    out=maxv[:, l:l + 1], in_=nt[:, l, :],
                    axis=mybir.AxisListType.X, op=MAX)
            # [c, l] -> [1, l, c]; broadcast to all partitions via K=1 matmul
            mvt = pmax.tile([1, 4, P], F32, tag="mvt")
            for l in range(4):
                nc.tensor.transpose(mvt[:, l, :], maxv[:, l:l + 1], ident)
            mvts = med.tile([1, 4, P], F32, tag="mvts")
            nc.scalar.copy(out=mvts, in_=mvt)
            maxB = pmax.tile([P, 4, C], F32, tag="maxB")  # [p(all), l, c]
            for l in range(4):
                nc.tensor.matmul(
                    maxB[:, l, :], onesrow, mvts[:, l, :],
                    start=True, stop=True,
                )

            # winner mask (exact equality; unique winner)
            mask = med.tile([P, A, 4, C], F32, tag="mask")
            nc.vector.tensor_tensor(
                mask[:, :, 0, :],
                X2[:, :, 0, :],
                maxB[:, 0:1, :].broadcast_to([P, A, C]),
                EQ,
            )
            nc.vector.tensor_tensor(
                mask[:, :, 1:4, :],
                norms,
                maxB[:, 1:4, :].unsqueeze(1).broadcast_to([P, A, 3, C]),
                EQ,
            )

            # select winner values in place: X *= mask[l(i)]
            for l, (s, e) in enumerate(BLOCKS):
                nc.vector.tensor_tensor(
                    X[:, :, s:e, :],
                    X[:, :, s:e, :],
                    mask[:, :, l, :].unsqueeze(2).broadcast_to([P, A, e - s, C]),
                    MULT,
                )

            # sum over n (partitions via PE, chunks via PSUM accumulation)
            Xf = X.rearrange("p a i c -> p a (i c)")
            ob = outp.tile([1, IRR * C], F32, tag="ob")
            for h in range(2):
                ps = pout.tile([1, 2, 512], F32, tag="ps")
                for kk in range(2):
                    k = h * 2 + kk
                    for a in range(A):
                        nc.tensor.matmul(
                            ps[:, kk, :],
                            ones,
                            Xf[:, a, k * 512:(k + 1) * 512],
                            start=(a == 0),
                            stop=(a == A - 1),
                        )
                nc.scalar.copy(out=ob[:, h * 1024:(h + 1) * 1024],
                               in_=ps.rearrange("m k f -> m (k f)"))
            nc.sync.dma_start(out=out_v[:, b * IRR * C:(b + 1) * IRR * C], in_=ob)

    nc.compile()
    return nc


def kernel(x: np.ndarray, i2l: np.ndarray | None = None) -> np.ndarray:
    x = np.ascontiguousarray(np.asarray(x), dtype=np.float32)
    assert x.shape == (B_FULL, N, IRR, C), x.shape

    if "nc" not in _cache:
        _cache["nc"] = _build_bass()
    nc = _cache["nc"]

    from concourse.bass_utils import run_bass_kernel_spmd

    in_maps = [{"x": x[i * B:(i + 1) * B]} for i in range(N_CORES)]
    res = run_bass_kernel_spmd(nc, in_maps, list(range(N_CORES)))
    out = np.concatenate([res.results[i]["out"] for i in range(N_CORES)], axis=0)
    return out


if __name__ == "__main__":
    xs = np.random.randn(B_FULL, N, IRR, C).astype(np.float32)
    o = kernel(xs)
    print("out", o.shape, o.dtype)
